# revision 55
# baseline (speedup 1.0000x reference)
"""MultiHeadAttention kernel for 8x TRN2 NeuronCores.

The reference module's einsum reduces the attention tensor over BOTH the
query and key axes (attn_mass = sum_{q,k} softmax(logits)_k), and softmax
rows sum to 1, so attn_mass == Lq exactly for every (batch, head).  The
whole computation collapses to a single dense GEMM after folding the
(block-diagonal) per-head V-projection into the output projection:

    out = V_flat @ W_eff + b_eff          (4096 x 1024) @ (1024 x 1024)
    W_eff[h*hd+a, n] = Lq * sum_b Wv[b, a] * Wo[n, h*hd+b]
    b_eff[n]         = Lq * sum_{h,b} Wo[n, h*hd+b] * bv[b] + bo[n]

Row-sharded across 8 cores (512 rows each), computed TRANSPOSED so the
bias is a per-partition scalar folded into the PSUM eviction.  All
operands stream as bf16 (tolerance is 2e-2; bf16 end-to-end lands at
~2.6e-3), which halves HBM traffic vs fp32 and runs the PE at 1 row/cycle.

Schedule (per core), tuned from NTFF traces (~180-240 B/ns per HWDGE
queue when both stream, ~1.4us DGE start + 0.9us completion-semaphore
latency, ~0.6us sequencer time per dma_start):
  * a tiny fast-start DMA [W0k0 | X0] leads the sync queue and unblocks
    the first real matmul ~4.5us in; bf16 junk matmuls on memset data
    keep the PE continuously busy before that (HAM clock gate + p-state
    ramp: 2.4GHz needs ~3us of UNINTERRUPTED busy - any idle resets it);
  * X slabs stream on sync right behind the head (they gate the bank-0/1
    sweep); W banks 1-3 ride the scalar queue behind bank0's W remainder,
    W4-7 fill the queue tails;
  * banks 0 and 1 interleave through the X-arrival window with junk
    fillers absorbing stream jitter; banks 2-7 then run back-to-back,
    each retiring with a bias-fused eviction + output DMA (alternating
    queues) that overlap the remaining compute;
  * the last bank's eviction runs in halves on TWO engines (DVE + Act)
    with output DMAs on both queues, minimizing the post-matmul tail.
"""

import numpy as np
import ml_dtypes

import concourse.bass as bass
import concourse.bacc as bacc
import concourse.mybir as mybir
from concourse.tile import TileContext
from concourse.bass_utils import run_bass_kernel_spmd

N_CORES = 8
E = 1024            # embed dim == d_model
H, HD = 16, 64      # heads, head dim
ROWS = 4096         # N * L = 2 * 2048
RPC = ROWS // N_CORES   # rows per core = 512
P = 128             # SBUF partitions
KT = E // P         # 8 contraction slabs
JT = E // P         # 8 output-column banks

# -- tuning knobs ------------------------------------------------------
N_WARM = 7          # junk matmuls before the first real matmul
JF = 512            # junk matmul free dim
JF2 = 256           # filler junk free dim (inside the X-gated phase)

BF16 = ml_dtypes.bfloat16

_NC_CACHE = {}
LAST_RESULTS = None  # BassKernelResults of the most recent device run


def _build():
    f32 = mybir.dt.float32
    bf = mybir.dt.bfloat16
    nc = bacc.Bacc(None, target_bir_lowering=False)

    # hd = [W0k0 | X0] fast-start block; wc0r = W0 k=1..7; xsp = X slabs
    # 1..7 packed; wcp = W banks 1..7 packed; bias per-bank per-partition.
    hd = nc.declare_dram_parameter("hd", [P, P + RPC], bf, isOutput=False)
    wc0r = nc.declare_dram_parameter("wc0r", [P, (KT - 1) * P], bf, isOutput=False)
    xsp = nc.declare_dram_parameter("xsp", [P, (KT - 1) * RPC], bf, isOutput=False)
    wcp = nc.declare_dram_parameter("wcp", [P, (JT - 1) * E], bf, isOutput=False)
    bias = nc.declare_dram_parameter("bias", [P, JT], f32, isOutput=False)
    outp = nc.declare_dram_parameter("outp", [P, JT * RPC], bf, isOutput=True)

    with TileContext(nc) as tc:
        with (
            tc.tile_pool(name="ip", bufs=1) as ip,
            tc.tile_pool(name="pp", bufs=1, space="PSUM") as pp,
            tc.tile_pool(name="op", bufs=1) as op,
        ):
            # junk-warm operands come from the framework's const-1.0 AP via
            # 0-stride broadcast: no DMA and no memset dependency, so the PE
            # warm-up (HAM gate + p-state ramp - nonzero data required)
            # starts right at kernel-body entry.
            cap = nc.const_aps.aps[(bf, 1.0)]
            cb_l = cap.broadcast_to([P, P])
            cb_r = {JF: cap.broadcast_to([P, JF]), JF2: cap.broadcast_to([P, JF2])}

            hd_t = ip.tile([P, P + RPC], bf, name="hd", tag="hd")
            wc0r_t = ip.tile([P, (KT - 1) * P], bf, name="wc0r", tag="wc0r")
            xsp_t = ip.tile([P, (KT - 1) * RPC], bf, name="xsp", tag="xsp")
            wcp_t = ip.tile([P, (JT - 1) * E], bf, name="wcp", tag="wcp")
            bias_t = ip.tile([P, JT], f32, name="bias", tag="bias")

            def xs_chunk(eng, a, b):   # X slabs a..b-1 (1-based slabs)
                eng.dma_start(
                    out=xsp_t[:, (a - 1) * RPC:(b - 1) * RPC],
                    in_=xsp[:, (a - 1) * RPC:(b - 1) * RPC],
                )

            def w_chunk(eng, a, b):    # W banks a..b-1 (1-based banks)
                eng.dma_start(
                    out=wcp_t[:, (a - 1) * E:(b - 1) * E],
                    in_=wcp[:, (a - 1) * E:(b - 1) * E],
                )

            nc.sync.dma_start(out=hd_t[:], in_=hd[:, :])
            xs_chunk(nc.sync, 1, 3)
            xs_chunk(nc.sync, 3, 4)
            xs_chunk(nc.sync, 4, 6)
            xs_chunk(nc.sync, 6, 8)
            w_chunk(nc.sync, 6, 8)
            nc.scalar.dma_start(out=wc0r_t[:], in_=wc0r[:, :])
            nc.scalar.dma_start(out=bias_t[:], in_=bias[:, :])
            w_chunk(nc.scalar, 1, 2)
            w_chunk(nc.scalar, 2, 3)
            w_chunk(nc.scalar, 3, 4)
            w_chunk(nc.scalar, 4, 6)

            ps = [
                pp.tile([P, RPC], f32, name=f"ps{j}", tag=f"ps{j}")
                for j in range(JT)
            ]

            def junk(i, f=JF2):
                nc.tensor.matmul(
                    ps[i % JT][:, 0:f],
                    cb_l,
                    cb_r[f],
                    start=True,
                    stop=True,
                )

            # PE warm-up on nonzero bf16 data, starting right after the
            # preamble so the HAM clock gate / p-state ramp is underway
            # before the first real matmul.
            for i in range(N_WARM):
                junk(i, JF)

            def lhsT(j, k):
                if j == 0:
                    if k == 0:
                        return hd_t[:, 0:P]
                    return wc0r_t[:, (k - 1) * P:k * P]
                return wcp_t[:, (j - 1) * E + k * P:(j - 1) * E + (k + 1) * P]

            def rhs(k):
                if k == 0:
                    return hd_t[:, P:P + RPC]
                return xsp_t[:, (k - 1) * RPC:k * RPC]

            ob = op.tile([P, JT * RPC], bf, name="ob", tag="ob")

            def mm(j, k):
                nc.tensor.matmul(
                    ps[j], lhsT(j, k), rhs(k),
                    start=(k == 0), stop=(k == KT - 1),
                )

            def evict(j):
                # alternate output queues so neither engine's dispatch train
                # backs up behind the other banks' output DMAs
                o = ob[:, j * RPC:(j + 1) * RPC]
                eng = nc.sync if j % 2 == 0 else nc.scalar
                nc.vector.tensor_scalar_add(o, ps[j], bias_t[:, j:j + 1])
                eng.dma_start(out=outp[:, j * RPC:(j + 1) * RPC], in_=o)

            def evict7(j):
                # final eviction split across TWO engines (DVE + Act) with
                # output DMAs on both queues; asymmetric split (DVE is
                # ~1.9ns/col, Act ~2.6ns/col) equalizes the two tail chains.
                hh = 288
                o = ob[:, j * RPC:(j + 1) * RPC]
                nc.vector.tensor_scalar_add(
                    o[:, 0:hh], ps[j][:, 0:hh], bias_t[:, j:j + 1]
                )
                nc.sync.dma_start(
                    out=outp[:, j * RPC:j * RPC + hh], in_=o[:, 0:hh]
                )
                nc.scalar.activation(
                    o[:, hh:RPC],
                    ps[j][:, hh:RPC],
                    mybir.ActivationFunctionType.Identity,
                    bias=bias_t[:, j:j + 1],
                )
                nc.scalar.dma_start(
                    out=outp[:, j * RPC + hh:(j + 1) * RPC],
                    in_=o[:, hh:RPC],
                )

            # X-gated phase: banks 0 and 1 interleaved, k groups following
            # the X chunk arrival order, with junk fillers absorbing stream
            # jitter so the PE p-state ramp never resets.  Banks 2-7 follow
            # back-to-back (compute-bound), each retiring with an eviction +
            # output DMA that overlap the remaining compute.
            fb = [2, 3, 4, 5, 6, 7]   # filler-target banks (not started yet)

            def fill(n):
                for _ in range(n):
                    junk(fb[fill.i % len(fb)])
                    fill.i += 1
            fill.i = 0

            mm(0, 0)
            fill(7)
            mm(0, 1)
            fill(1)
            mm(0, 2)
            fill(1)
            mm(0, 3)
            fill(1)
            mm(1, 0)
            mm(1, 1)
            fill(1)
            mm(1, 2)
            mm(1, 3)
            fill(2)
            mm(0, 4)
            mm(0, 5)
            fill(1)
            mm(1, 4)
            mm(1, 5)
            fill(3)
            mm(0, 6)
            mm(0, 7)
            mm(1, 6)
            mm(1, 7)
            evict(0)
            evict(1)
            for j in range(2, JT):
                for k in range(KT):
                    mm(j, k)
                if j < JT - 1:
                    evict(j)
                else:
                    evict7(j)
    nc.compile()
    return nc


def _get_nc():
    if "nc" not in _NC_CACHE:
        _NC_CACHE["nc"] = _build()
    return _NC_CACHE["nc"]


def _prep_in_maps(V, Wv, bv, Wo, bo, lq):
    Wv64 = np.asarray(Wv, np.float64)
    Wo64 = np.asarray(Wo, np.float64)
    bv64 = np.asarray(bv, np.float64)
    bo64 = np.asarray(bo, np.float64)

    # Fold per-head V-projection + output projection + attention mass (== Lq).
    Wo_r = Wo64.reshape(E, H, HD)                       # [n, h, b]
    W_eff = lq * np.einsum("ba,nhb->han", Wv64, Wo_r, optimize=True)
    W_eff = W_eff.reshape(E, E).astype(np.float32)      # [k, n]
    b_eff = (lq * np.einsum("nhb,b->n", Wo_r, bv64) + bo64).astype(np.float32)

    # wc_all[p, j*E + k*P + c] = W_eff[k*P + p, j*P + c]  (lhsT blocks)
    wc_all = np.ascontiguousarray(
        W_eff.reshape(KT, P, JT, P).transpose(1, 2, 0, 3).reshape(P, JT * E)
    ).astype(BF16)
    bias_blk = np.ascontiguousarray(b_eff.reshape(JT, P).T)   # [p, j] f32

    X = np.asarray(V, dtype=np.float32).reshape(ROWS, E).astype(BF16)
    wc0r = np.ascontiguousarray(wc_all[:, P:E])
    wcp = np.ascontiguousarray(wc_all[:, E:])
    in_maps = []
    for i in range(N_CORES):
        xsT = np.ascontiguousarray(X[i * RPC:(i + 1) * RPC, :].T)  # [E, RPC]
        hd_i = np.empty((P, P + RPC), BF16)
        hd_i[:, :P] = wc_all[:, :P]
        hd_i[:, P:] = xsT[0:P, :]
        xsp_i = np.ascontiguousarray(
            xsT.reshape(KT, P, RPC)[1:].transpose(1, 0, 2).reshape(P, (KT - 1) * RPC)
        )
        in_maps.append(
            {"hd": hd_i, "wc0r": wc0r, "xsp": xsp_i, "wcp": wcp, "bias": bias_blk}
        )
    return in_maps


def kernel(Q, K, V, Wq, bq, Wk, bk, Wv, bv, Wo, bo, **_unused):
    global LAST_RESULTS
    n, L, e = np.asarray(V).shape
    lq = float(np.asarray(Q).shape[1])
    in_maps = _prep_in_maps(V, Wv, bv, Wo, bo, lq)
    nc = _get_nc()
    LAST_RESULTS = run_bass_kernel_spmd(nc, in_maps, list(range(N_CORES)))
    parts = []
    for i in range(N_CORES):
        outp = LAST_RESULTS.results[i]["outp"]          # [P, JT*RPC] bf16
        oT = outp.reshape(P, JT, RPC).transpose(1, 0, 2).reshape(E, RPC)
        parts.append(np.ascontiguousarray(oT.T).astype(np.float32))
    out = np.concatenate(parts, axis=0)
    return np.ascontiguousarray(out).reshape(n, L, E)


# revision 57
# speedup vs baseline: 1.0680x; 1.0680x over previous
"""MultiHeadAttention kernel for 8x TRN2 NeuronCores.

The reference module's einsum reduces the attention tensor over BOTH the
query and key axes (attn_mass = sum_{q,k} softmax(logits)_k), and softmax
rows sum to 1, so attn_mass == Lq exactly for every (batch, head).  The
whole computation collapses to a single dense GEMM after folding the
(block-diagonal) per-head V-projection into the output projection:

    out = V_flat @ W_eff + b_eff          (4096 x 1024) @ (1024 x 1024)
    W_eff[h*hd+a, n] = Lq * sum_b Wv[b, a] * Wo[n, h*hd+b]
    b_eff[n]         = Lq * sum_{h,b} Wo[n, h*hd+b] * bv[b] + bo[n]

Row-sharded across 8 cores (512 rows each), computed TRANSPOSED so the
bias is a per-partition scalar folded into the PSUM eviction.  All
operands stream as bf16 (tolerance is 2e-2; bf16 end-to-end lands at
~2.6e-3), which halves HBM traffic vs fp32 and runs the PE at 1 row/cycle.

Schedule (per core), tuned from NTFF traces (~180-240 B/ns per HWDGE
queue when both stream, ~1.4us DGE start + 0.9us completion-semaphore
latency, ~0.6us sequencer time per dma_start):
  * a tiny fast-start DMA [W0k0 | X0] leads the sync queue and unblocks
    the first real matmul ~4.5us in; bf16 junk matmuls on memset data
    keep the PE continuously busy before that (HAM clock gate + p-state
    ramp: 2.4GHz needs ~3us of UNINTERRUPTED busy - any idle resets it);
  * X slabs stream on sync right behind the head (they gate the bank-0/1
    sweep); W banks 1-3 ride the scalar queue behind bank0's W remainder,
    W4-7 fill the queue tails;
  * banks 0 and 1 interleave through the X-arrival window with junk
    fillers absorbing stream jitter; banks 2-7 then run back-to-back,
    each retiring with a bias-fused eviction + output DMA (alternating
    queues) that overlap the remaining compute;
  * the last bank's eviction runs in halves on TWO engines (DVE + Act)
    with output DMAs on both queues, minimizing the post-matmul tail.
"""

import numpy as np
import ml_dtypes

import concourse.bass as bass
import concourse.bacc as bacc
import concourse.mybir as mybir
from concourse.tile import TileContext
from concourse.bass_utils import run_bass_kernel_spmd

N_CORES = 8
E = 1024            # embed dim == d_model
H, HD = 16, 64      # heads, head dim
ROWS = 4096         # N * L = 2 * 2048
RPC = ROWS // N_CORES   # rows per core = 512
P = 128             # SBUF partitions
KT = E // P         # 8 contraction slabs
JT = E // P         # 8 output-column banks

# -- tuning knobs ------------------------------------------------------
N_WARM = 7          # junk matmuls before the first real matmul
JF = 512            # junk matmul free dim
JF2 = 256           # filler junk free dim (inside the X-gated phase)

BF16 = ml_dtypes.bfloat16

_NC_CACHE = {}
LAST_RESULTS = None  # BassKernelResults of the most recent device run


def _build():
    f32 = mybir.dt.float32
    bf = mybir.dt.bfloat16
    nc = bacc.Bacc(None, target_bir_lowering=False)

    # hd = [W0k0 | X0] fast-start block; wc0r = W0 k=1..7; xsp = X slabs
    # 1..7 packed; wcp = W banks 1..7 packed; bias per-bank per-partition.
    hd = nc.declare_dram_parameter("hd", [P, P + RPC], bf, isOutput=False)
    wc0r = nc.declare_dram_parameter("wc0r", [P, (KT - 1) * P], bf, isOutput=False)
    xsp = nc.declare_dram_parameter("xsp", [P, (KT - 1) * RPC], bf, isOutput=False)
    wcp = nc.declare_dram_parameter("wcp", [P, (JT - 1) * E], bf, isOutput=False)
    bias = nc.declare_dram_parameter("bias", [P, JT], f32, isOutput=False)
    outp = nc.declare_dram_parameter("outp", [P, JT * RPC], bf, isOutput=True)

    with TileContext(nc) as tc:
        with (
            tc.tile_pool(name="ip", bufs=1) as ip,
            tc.tile_pool(name="pp", bufs=1, space="PSUM") as pp,
            tc.tile_pool(name="op", bufs=1) as op,
        ):
            # junk-warm operands come from the framework's const-1.0 AP via
            # 0-stride broadcast: no DMA and no memset dependency, so the PE
            # warm-up (HAM gate + p-state ramp - nonzero data required)
            # starts right at kernel-body entry.
            cap = nc.const_aps.aps[(bf, 1.0)]
            cb_l = cap.broadcast_to([P, P])
            cb_r = {JF: cap.broadcast_to([P, JF]), JF2: cap.broadcast_to([P, JF2])}

            hd_t = ip.tile([P, P + RPC], bf, name="hd", tag="hd")
            wc0r_t = ip.tile([P, (KT - 1) * P], bf, name="wc0r", tag="wc0r")
            xsp_t = ip.tile([P, (KT - 1) * RPC], bf, name="xsp", tag="xsp")
            wcp_t = ip.tile([P, (JT - 1) * E], bf, name="wcp", tag="wcp")
            bias_t = ip.tile([P, JT], f32, name="bias", tag="bias")

            def xs_chunk(eng, a, b):   # X slabs a..b-1 (1-based slabs)
                eng.dma_start(
                    out=xsp_t[:, (a - 1) * RPC:(b - 1) * RPC],
                    in_=xsp[:, (a - 1) * RPC:(b - 1) * RPC],
                )

            def w_chunk(eng, a, b):    # W banks a..b-1 (1-based banks)
                eng.dma_start(
                    out=wcp_t[:, (a - 1) * E:(b - 1) * E],
                    in_=wcp[:, (a - 1) * E:(b - 1) * E],
                )

            nc.sync.dma_start(out=hd_t[:], in_=hd[:, :])
            xs_chunk(nc.sync, 1, 3)
            xs_chunk(nc.sync, 3, 4)
            xs_chunk(nc.sync, 4, 6)
            xs_chunk(nc.sync, 6, 8)
            w_chunk(nc.sync, 6, 8)
            nc.scalar.dma_start(out=wc0r_t[:], in_=wc0r[:, :])
            nc.scalar.dma_start(out=bias_t[:], in_=bias[:, :])
            w_chunk(nc.scalar, 1, 2)
            w_chunk(nc.scalar, 2, 3)
            w_chunk(nc.scalar, 3, 4)
            w_chunk(nc.scalar, 4, 6)

            ps = [
                pp.tile([P, RPC], f32, name=f"ps{j}", tag=f"ps{j}")
                for j in range(JT)
            ]

            def junk(i, f=JF2):
                nc.tensor.matmul(
                    ps[i % JT][:, 0:f],
                    cb_l,
                    cb_r[f],
                    start=True,
                    stop=True,
                )

            # PE warm-up on nonzero bf16 data, starting right after the
            # preamble so the HAM clock gate / p-state ramp is underway
            # before the first real matmul.
            for i in range(N_WARM):
                junk(i, JF)

            def lhsT(j, k):
                if j == 0:
                    if k == 0:
                        return hd_t[:, 0:P]
                    return wc0r_t[:, (k - 1) * P:k * P]
                return wcp_t[:, (j - 1) * E + k * P:(j - 1) * E + (k + 1) * P]

            def rhs(k):
                if k == 0:
                    return hd_t[:, P:P + RPC]
                return xsp_t[:, (k - 1) * RPC:k * RPC]

            ob = op.tile([P, JT * RPC], bf, name="ob", tag="ob")

            def mm(j, k):
                nc.tensor.matmul(
                    ps[j], lhsT(j, k), rhs(k),
                    start=(k == 0), stop=(k == KT - 1),
                )

            def evict(j):
                # alternate output queues so neither engine's dispatch train
                # backs up behind the other banks' output DMAs
                o = ob[:, j * RPC:(j + 1) * RPC]
                eng = nc.sync if j % 2 == 0 else nc.scalar
                nc.vector.tensor_scalar_add(o, ps[j], bias_t[:, j:j + 1])
                eng.dma_start(out=outp[:, j * RPC:(j + 1) * RPC], in_=o)

            def evict7(j):
                # final eviction split across TWO engines (DVE + Act) with
                # output DMAs on both queues; asymmetric split (DVE is
                # ~1.9ns/col, Act ~2.6ns/col) equalizes the two tail chains.
                hh = 288
                o = ob[:, j * RPC:(j + 1) * RPC]
                nc.vector.tensor_scalar_add(
                    o[:, 0:hh], ps[j][:, 0:hh], bias_t[:, j:j + 1]
                )
                nc.sync.dma_start(
                    out=outp[:, j * RPC:j * RPC + hh], in_=o[:, 0:hh]
                )
                nc.scalar.activation(
                    o[:, hh:RPC],
                    ps[j][:, hh:RPC],
                    mybir.ActivationFunctionType.Identity,
                    bias=bias_t[:, j:j + 1],
                )
                nc.scalar.dma_start(
                    out=outp[:, j * RPC + hh:(j + 1) * RPC],
                    in_=o[:, hh:RPC],
                )

            # X-gated phase: banks 0 and 1 interleaved, k groups following
            # the X chunk arrival order, with junk fillers absorbing stream
            # jitter so the PE p-state ramp never resets.  Banks 2-7 follow
            # back-to-back (compute-bound), each retiring with an eviction +
            # output DMA that overlap the remaining compute.
            fb = [2, 3, 4, 5, 6, 7]   # filler-target banks (not started yet)

            def fill(n):
                for _ in range(n):
                    junk(fb[fill.i % len(fb)])
                    fill.i += 1
            fill.i = 0

            mm(0, 0)
            fill(5)
            mm(0, 1)
            fill(1)
            mm(0, 2)
            fill(1)
            mm(0, 3)
            fill(1)
            mm(1, 0)
            mm(1, 1)
            fill(1)
            mm(1, 2)
            mm(1, 3)
            fill(2)
            mm(0, 4)
            mm(0, 5)
            fill(1)
            mm(1, 4)
            mm(1, 5)
            fill(2)
            mm(0, 6)
            mm(0, 7)
            mm(1, 6)
            mm(1, 7)
            evict(0)
            evict(1)
            for j in range(2, JT):
                for k in range(KT):
                    mm(j, k)
                if j < JT - 1:
                    evict(j)
                else:
                    evict7(j)
    nc.compile()
    return nc


def _get_nc():
    if "nc" not in _NC_CACHE:
        _NC_CACHE["nc"] = _build()
    return _NC_CACHE["nc"]


def _prep_in_maps(V, Wv, bv, Wo, bo, lq):
    Wv64 = np.asarray(Wv, np.float64)
    Wo64 = np.asarray(Wo, np.float64)
    bv64 = np.asarray(bv, np.float64)
    bo64 = np.asarray(bo, np.float64)

    # Fold per-head V-projection + output projection + attention mass (== Lq).
    Wo_r = Wo64.reshape(E, H, HD)                       # [n, h, b]
    W_eff = lq * np.einsum("ba,nhb->han", Wv64, Wo_r, optimize=True)
    W_eff = W_eff.reshape(E, E).astype(np.float32)      # [k, n]
    b_eff = (lq * np.einsum("nhb,b->n", Wo_r, bv64) + bo64).astype(np.float32)

    # wc_all[p, j*E + k*P + c] = W_eff[k*P + p, j*P + c]  (lhsT blocks)
    wc_all = np.ascontiguousarray(
        W_eff.reshape(KT, P, JT, P).transpose(1, 2, 0, 3).reshape(P, JT * E)
    ).astype(BF16)
    bias_blk = np.ascontiguousarray(b_eff.reshape(JT, P).T)   # [p, j] f32

    X = np.asarray(V, dtype=np.float32).reshape(ROWS, E).astype(BF16)
    wc0r = np.ascontiguousarray(wc_all[:, P:E])
    wcp = np.ascontiguousarray(wc_all[:, E:])
    in_maps = []
    for i in range(N_CORES):
        xsT = np.ascontiguousarray(X[i * RPC:(i + 1) * RPC, :].T)  # [E, RPC]
        hd_i = np.empty((P, P + RPC), BF16)
        hd_i[:, :P] = wc_all[:, :P]
        hd_i[:, P:] = xsT[0:P, :]
        xsp_i = np.ascontiguousarray(
            xsT.reshape(KT, P, RPC)[1:].transpose(1, 0, 2).reshape(P, (KT - 1) * RPC)
        )
        in_maps.append(
            {"hd": hd_i, "wc0r": wc0r, "xsp": xsp_i, "wcp": wcp, "bias": bias_blk}
        )
    return in_maps


def kernel(Q, K, V, Wq, bq, Wk, bk, Wv, bv, Wo, bo, **_unused):
    global LAST_RESULTS
    n, L, e = np.asarray(V).shape
    lq = float(np.asarray(Q).shape[1])
    in_maps = _prep_in_maps(V, Wv, bv, Wo, bo, lq)
    nc = _get_nc()
    LAST_RESULTS = run_bass_kernel_spmd(nc, in_maps, list(range(N_CORES)))
    parts = []
    for i in range(N_CORES):
        outp = LAST_RESULTS.results[i]["outp"]          # [P, JT*RPC] bf16
        oT = outp.reshape(P, JT, RPC).transpose(1, 0, 2).reshape(E, RPC)
        parts.append(np.ascontiguousarray(oT.T).astype(np.float32))
    out = np.concatenate(parts, axis=0)
    return np.ascontiguousarray(out).reshape(n, L, E)
